# revision 1
# baseline (speedup 1.0000x reference)
"""Masked phase-locking value (PLV) kernel for Trainium2, 8 NeuronCores.

Math: out[b] = |sum_ij M_ij * exp(i*(a_bi - b_bj))| / max(sum(M), 1)
    real_b = sum_ij M_ij (cos a_bi cos b_bj + sin a_bi sin b_bj)
    imag_b = sum_ij M_ij (sin a_bi cos b_bj - cos a_bi sin b_bj)

Device decomposition (per core, Na sharded 8 ways -> 1024 rows each):
    acc[m, j] = sum_i W[i, m] * mask[i, j]     (TensorE; W = [ca^T | sa^T], m = 2B = 128)
    racc[m]   = sum_j acc[m, j] * CS[m, j]     (DVE mult, ACT accumulate; CS = [cb; sb])
    qacc[m]   = sum_j acc[m, j] * SW[m, j]     (SW = [sb; cb], partition-swap of CS)
real_b = sum_cores racc[b] + racc[64+b]; imag_b = sum_cores qacc[64+b] - qacc[b].
All bilinear in mask rows, so Na-shard partials just add; host does the tiny
fold + |z| / sum(M).

dtypes: mask is 0/1 -> exact in fp8e4 (1 byte, halves HBM traffic, full PE rate);
weights/CS fp16 (PE full rate); SW in fp8 (the imag side is an incoherent sum,
tiny vs the coherent real part, so fp8 there costs ~1e-5 extra error);
PSUM/epilogue fp32. End-to-end rel err ~2e-5.
Column groups are sized small-big-small: a small first group starts the PE
early, small last groups shorten the end-of-stream epilogue tail. Trig rides
the scalar HWDGE ring (doesn't queue behind masks); a PE warm-up burst during
the DMA lead-in defeats the HAM cold-clock penalty.
"""

import numpy as np

import concourse.bass as bass
import concourse.tile as tile
from concourse import bacc, mybir
from concourse.bass_utils import run_bass_kernel_spmd

B = 64
NA = 8192
NB = 8192
NCORES = 8
NASH = NA // NCORES          # mask rows per core
KCH = NASH // 128            # contraction chunks of 128 rows
NCH = 512                    # output columns per PSUM bank / matmul

# column group widths: small first (early PE start), small last (short tail)
GWS = [512, 1024, 1024, 1024, 1024, 1024, 1024, 512, 512, 256, 256]
assert sum(GWS) == NB and all(w % 256 == 0 for w in GWS)
NG = len(GWS)
GOFF = [sum(GWS[:i]) for i in range(NG)]

# trig upload pieces (scalar ring): first small so group 0's epilogue isn't gated
TP = [1024, 2048, 2560, 2560]
assert sum(TP) == NB
TPOFF = [sum(TP[:i]) for i in range(len(TP))]

F8 = mybir.dt.float8e4
F16 = mybir.dt.float16
F32 = mybir.dt.float32


def build_program() -> bass.Bass:
    nc = bacc.Bacc("TRN2")
    # concatenated per-group blocks, each contiguous [128, KCH, gw]
    mask_d = nc.dram_tensor("mask", [128 * KCH * NB], F8, kind="ExternalInput")
    w_d = nc.dram_tensor("w", [128, KCH, 2 * B], F16, kind="ExternalInput")
    cs_d = nc.dram_tensor("cs", [128, NB], F16, kind="ExternalInput")
    sw_d = nc.dram_tensor("sw", [128, NB], F8, kind="ExternalInput")
    out_d = nc.dram_tensor("out", [128, 2 * NG], F32, kind="ExternalOutput")

    copy_f = mybir.ActivationFunctionType.Copy

    with tile.TileContext(nc) as tc:
        with (
            tc.tile_pool(name="consts", bufs=1) as consts,
            tc.tile_pool(name="masks", bufs=NG) as masks,
            tc.tile_pool(name="scratch", bufs=3) as scratch,
            tc.tile_pool(name="junk", bufs=2) as junkp,
            tc.tile_pool(name="psum", bufs=3, space="PSUM") as psum_pool,
            tc.tile_pool(name="wups", bufs=1, space="PSUM") as wu_pool,
        ):
            w_sb = consts.tile([128, KCH, 2 * B], F16)
            nc.sync.dma_start(out=w_sb[:], in_=w_d[:])
            cs_sb = consts.tile([128, NB], F16)
            sw_sb = consts.tile([128, NB], F8)
            racc = consts.tile([128, 2 * NG], F32)

            # PE warm-up while the first mask group is in flight (HAM ramp)
            wu_ps = wu_pool.tile([128, 2 * B], F32)
            for r in range(16):
                nc.tensor.matmul(
                    out=wu_ps[:],
                    lhsT=w_sb[:, 0, :],
                    rhs=w_sb[:, 1, :],
                    start=(r == 0),
                    stop=(r == 15),
                )

            tp_emitted = 0
            for g in range(NG):
                off, gw = GOFF[g], GWS[g]
                gsl = slice(off, off + gw)
                mt = masks.tile([128, KCH, gw], F8, tag="mask")
                blk = 128 * KCH
                src = mask_d[off * blk : (off + gw) * blk].rearrange(
                    "(p k c) -> p k c", p=128, k=KCH
                )
                nc.sync.dma_start(out=mt[:], in_=src)
                # trig pieces on the scalar HWDGE ring, paced ahead of use
                while tp_emitted < len(TP) and TPOFF[tp_emitted] < off + gw:
                    tsl = slice(TPOFF[tp_emitted], TPOFF[tp_emitted] + TP[tp_emitted])
                    nc.scalar.dma_start(out=cs_sb[:, tsl], in_=cs_d[:, tsl])
                    nc.scalar.dma_start(out=sw_sb[:, tsl], in_=sw_d[:, tsl])
                    tp_emitted += 1

                ps = psum_pool.tile([128, gw], F32, tag="psum")
                for j0 in range(0, gw, NCH):
                    jsl = slice(j0, min(j0 + NCH, gw))
                    for k in range(KCH):
                        nc.tensor.matmul(
                            out=ps[:, jsl],
                            lhsT=w_sb[:, k, :],
                            rhs=mt[:, k, jsl],
                            start=(k == 0),
                            stop=(k == KCH - 1),
                        )
                rcol = g if g < 8 else 16 + (g - 8)
                qcol = 8 + g if g < 8 else 16 + (NG - 8) + (g - 8)
                pr = scratch.tile([128, gw], F32, tag="pr")
                nc.vector.tensor_mul(out=pr[:], in0=ps[:], in1=cs_sb[:, gsl])
                jr = junkp.tile([128, gw], F32, tag="junk")
                nc.scalar.activation(
                    out=jr[:], in_=pr[:], func=copy_f,
                    accum_out=racc[:, rcol : rcol + 1],
                )
                pi = scratch.tile([128, gw], F32, tag="pr")
                nc.vector.tensor_mul(out=pi[:], in0=ps[:], in1=sw_sb[:, gsl])
                ji = junkp.tile([128, gw], F32, tag="junk")
                nc.scalar.activation(
                    out=ji[:], in_=pi[:], func=copy_f,
                    accum_out=racc[:, qcol : qcol + 1],
                )
                if g == 7:
                    # groups 0-7 partials fly out while tail groups finish
                    nc.sync.dma_start(out=out_d[:, :16], in_=racc[:, :16])

            nc.sync.dma_start(out=out_d[:, 16:], in_=racc[:, 16:])
    nc.finalize()
    return nc


def prep_inputs(phases_a, phases_b, coupling_mask):
    pa = np.asarray(phases_a, dtype=np.float32)
    pb = np.asarray(phases_b, dtype=np.float32)
    ca, sa = np.cos(pa), np.sin(pa)
    cb, sb = np.cos(pb), np.sin(pb)
    cs = np.concatenate([cb, sb], axis=0).astype(np.float16)
    sw = np.concatenate([sb, cb], axis=0).astype(mybir.dt.np(F8))

    f8np = mybir.dt.np(F8)
    one_byte = np.array([1.0], f8np).view(np.uint8)[0]
    mask_u8 = (np.asarray(coupling_mask) != 0).astype(np.uint8) * one_byte

    in_maps = []
    for c in range(NCORES):
        rows = slice(c * NASH, (c + 1) * NASH)
        W = np.empty((NASH, 2 * B), np.float16)
        W[:, :B] = ca[:, rows].T
        W[:, B:] = sa[:, rows].T
        # [i = k*128 + p, m] -> [p, k, m]
        w_host = np.ascontiguousarray(W.reshape(KCH, 128, 2 * B).transpose(1, 0, 2))
        # per group: contiguous [p, k, c] block; blocks concatenated flat
        mr = mask_u8[rows].reshape(KCH, 128, NB)
        blocks = [
            np.ascontiguousarray(
                mr[:, :, GOFF[g] : GOFF[g] + GWS[g]].transpose(1, 0, 2)
            ).reshape(-1)
            for g in range(NG)
        ]
        m_host = np.concatenate(blocks).view(f8np)
        in_maps.append({"mask": m_host, "w": w_host, "cs": cs, "sw": sw})
    return in_maps


def combine(outs, coupling_mask):
    o = np.stack(outs).astype(np.float64)  # [NCORES, 128, 2*NG]
    nt = NG - 8
    r = o[:, :, :8].sum(axis=2) + o[:, :, 16 : 16 + nt].sum(axis=2)
    q = o[:, :, 8:16].sum(axis=2) + o[:, :, 16 + nt :].sum(axis=2)
    real = (r[:, :B] + r[:, B:]).sum(axis=0)
    imag = (q[:, B:] - q[:, :B]).sum(axis=0)
    n_pairs = max(float(np.asarray(coupling_mask).sum()), 1.0)
    return (np.sqrt(real * real + imag * imag) / n_pairs).astype(np.float32)


_prog_cache: list = []


def kernel(phases_a, phases_b, coupling_mask):
    in_maps = prep_inputs(phases_a, phases_b, coupling_mask)
    if not _prog_cache:
        _prog_cache.append(build_program())
    res = run_bass_kernel_spmd(_prog_cache[0], in_maps, core_ids=list(range(NCORES)))
    return combine([r["out"] for r in res.results], coupling_mask)



# revision 4
# speedup vs baseline: 1.2102x; 1.2102x over previous
"""Masked phase-locking value (PLV) kernel for Trainium2, 8 NeuronCores.

Math: out[b] = |sum_ij M_ij * exp(i*(a_bi - b_bj))| / max(sum(M), 1)
    real_b = sum_ij M_ij (ca_bi cb_bj + sa_bi sb_bj)
    imag_b = sum_ij M_ij (sa_bi cb_bj - ca_bi sb_bj)

Device decomposition (per core, Na sharded 8 ways -> 1024 mask rows each):
    Z[m, i] = sum_j CST[j, m] * maskT[j, i]      (TensorE, contract full Nb)
      where CST[j, m] = cb[m, j] for m<64, sb[m-64, j] for m>=64  (m = 2B = 128)
    racc[m] = sum_i Z[m, i] * W2[m, i]           (DVE fused mul+reduce)
    qacc[m] = sum_i Z[m, i] * W2S[m, i]
      W2[m,i]  = ca[m,i] | sa[m-64,i],  W2S[m,i] = sa[m,i] | -ca[m-64,i]
real_b = sum_cores racc[b] + racc[64+b]; imag_b = qacc[b] + qacc[64+b].

Contracting Nb (8192) on the PE and only Na/8 (1024) on the DVE makes the
epilogue 8x smaller than the W-stationary orientation. The matmul runs in
fp8 DoubleRow mode (2 fp8 weights per PE cell, 256-deep contraction per
matmul): 64 matmuls of N=512 instead of 128 -> ~15us of PE time, hidden
under the ~22us mask DMA (8MB/core at ~420GB/s measured). Mask is 0/1 ->
exact in fp8e4; trig in fp8e4 adds ~1e-3 rel err (coherent real part sums
errors as a random walk), well inside the 2e-2 gate. A PE warm-up burst on
a memset tile during the DMA lead-in defeats the HAM cold-clock penalty.
"""

import numpy as np

import concourse.bass as bass
import concourse.tile as tile
from concourse import bacc, mybir
from concourse.bass_utils import run_bass_kernel_spmd

B = 64
NA = 8192
NB = 8192
NCORES = 8
NASH = NA // NCORES          # mask rows (i) per core
NBLK = 2                     # i blocks per core (PSUM banks)
IBW = NASH // NBLK           # i columns per block = 512
NJC = NB // 256              # 32 DoubleRow matmuls per block (256-deep each)
JCKP = 2 * NJC               # 64 weight half-chunks of 128 j each
NCHUNK = 4                   # mask DMA chunks per i-block (1MB each)
QW = JCKP // NCHUNK          # 16 jckp per DMA chunk

F8 = mybir.dt.float8e4
F16 = mybir.dt.float16
F32 = mybir.dt.float32
DR = mybir.MatmulPerfMode.DoubleRow
MUL = mybir.AluOpType.mult
ADD = mybir.AluOpType.add

N_WARM = 7                   # cold-rate N=512 matmuls ~= 3us HAM warmup


def build_program() -> bass.Bass:
    nc = bacc.Bacc("TRN2")
    # per-chunk contiguous [p, 16, 512] blocks, chunk-major (ib, q)
    mask_d = nc.dram_tensor("mask", [128 * NBLK * JCKP * IBW], F8, kind="ExternalInput")
    cst_d = nc.dram_tensor("cst", [128 * JCKP * 128], F8, kind="ExternalInput")
    w2_d = nc.dram_tensor("w2", [128, NASH], F16, kind="ExternalInput")
    w2s_d = nc.dram_tensor("w2s", [128, NASH], F16, kind="ExternalInput")
    out_d = nc.dram_tensor("out", [128, 2 * NBLK], F32, kind="ExternalOutput")

    with tile.TileContext(nc) as tc:
        with (
            tc.tile_pool(name="consts", bufs=1) as consts,
            tc.tile_pool(name="scratch", bufs=4) as scratch,
            tc.tile_pool(name="psum", bufs=NBLK, space="PSUM") as psum_pool,
            tc.tile_pool(name="wups", bufs=1, space="PSUM") as wu_pool,
        ):
            # PE warm-up source: memset, no DMA dependency
            wu_in = consts.tile([128, 512], F16)
            nc.vector.memset(wu_in[:], 0.25)

            # trig + epilogue weights ride the scalar HWDGE ring
            cst_sb = consts.tile([128, JCKP, 128], F8)
            nc.scalar.dma_start(
                out=cst_sb[:],
                in_=cst_d[:].rearrange("(p k m) -> p k m", p=128, k=JCKP),
            )
            w2_sb = consts.tile([128, NASH], F16)
            nc.scalar.dma_start(out=w2_sb[:], in_=w2_d[:])
            w2s_sb = consts.tile([128, NASH], F16)
            nc.scalar.dma_start(out=w2s_sb[:], in_=w2s_d[:])

            # mask chunks on the sync ring, i-block-major so block 0 can
            # finish + fold while block 1 still streams
            mt = consts.tile([128, NBLK, JCKP, IBW], F8)
            blk = 128 * QW * IBW
            for ib in range(NBLK):
                for q in range(NCHUNK):
                    off = (ib * NCHUNK + q) * blk
                    nc.sync.dma_start(
                        out=mt[:, ib, q * QW : (q + 1) * QW, :],
                        in_=mask_d[off : off + blk].rearrange(
                            "(p q i) -> p q i", p=128, q=QW
                        ),
                    )

            # HAM warm-up while the first chunks are in flight
            wu_ps = wu_pool.tile([128, 512], F32)
            for r in range(N_WARM):
                nc.tensor.matmul(
                    out=wu_ps[:],
                    lhsT=wu_in[:, 0:128],
                    rhs=wu_in[:],
                    start=(r == 0),
                    stop=(r == N_WARM - 1),
                )

            racc = consts.tile([128, 2 * NBLK], F32)
            copy_f = mybir.ActivationFunctionType.Copy
            for ib in range(NBLK):
                ps = psum_pool.tile([128, IBW], F32, tag="psum")
                for jc in range(NJC):
                    nc.tensor.matmul(
                        out=ps[:],
                        lhsT=cst_sb[:, 2 * jc : 2 * jc + 2, :],
                        rhs=mt[:, ib, 2 * jc : 2 * jc + 2, :],
                        start=(jc == 0),
                        stop=(jc == NJC - 1),
                        perf_mode=DR,
                    )
                isl = slice(ib * IBW, (ib + 1) * IBW)
                pr = scratch.tile([128, IBW], F32, tag="pr")
                nc.vector.tensor_mul(out=pr[:], in0=ps[:], in1=w2_sb[:, isl])
                jr = scratch.tile([128, IBW], F32, tag="junk")
                nc.scalar.activation(
                    out=jr[:], in_=pr[:], func=copy_f,
                    accum_out=racc[:, ib : ib + 1],
                )
                pi = scratch.tile([128, IBW], F32, tag="pr")
                nc.vector.tensor_mul(out=pi[:], in0=ps[:], in1=w2s_sb[:, isl])
                ji = scratch.tile([128, IBW], F32, tag="junk")
                nc.scalar.activation(
                    out=ji[:], in_=pi[:], func=copy_f,
                    accum_out=racc[:, NBLK + ib : NBLK + ib + 1],
                )

            nc.scalar.dma_start(out=out_d[:], in_=racc[:])
    nc.finalize()
    return nc


def prep_inputs(phases_a, phases_b, coupling_mask):
    pa = np.asarray(phases_a, dtype=np.float32)
    pb = np.asarray(phases_b, dtype=np.float32)
    ca, sa = np.cos(pa), np.sin(pa)    # (B, Na)
    cb, sb = np.cos(pb), np.sin(pb)    # (B, Nb)
    f8np = mybir.dt.np(F8)

    # CST[j, m]: cb for m<64, sb for m>=64; tile layout [p, jc, kp, m],
    # j = jc*256 + kp*128 + p
    cst = np.concatenate([cb, sb], axis=0).T.astype(f8np)   # (Nb, 128)
    cst_host = np.ascontiguousarray(
        cst.reshape(NJC, 2, 128, 128).transpose(2, 0, 1, 3)
    ).reshape(-1)

    one_byte = np.array([1.0], f8np).view(np.uint8)[0]
    mask_u8 = (np.asarray(coupling_mask) != 0).astype(np.uint8) * one_byte

    in_maps = []
    for c in range(NCORES):
        rows = slice(c * NASH, (c + 1) * NASH)
        # maskT [j, i] -> [p, ib, jckp, i] -> chunk-major [ib, q, p, r, i]
        mT = np.ascontiguousarray(mask_u8[rows].T)          # (Nb, NASH)
        A = mT.reshape(NJC, 2, 128, NBLK, IBW).transpose(2, 3, 0, 1, 4)
        A = A.reshape(128, NBLK, JCKP, IBW)
        m_host = (
            A.reshape(128, NBLK, NCHUNK, QW, IBW)
            .transpose(1, 2, 0, 3, 4)
            .reshape(-1)
            .view(f8np)
        )

        w2 = np.empty((128, NASH), np.float16)
        w2[:B] = ca[:, rows]
        w2[B:] = sa[:, rows]
        w2s = np.empty((128, NASH), np.float16)
        w2s[:B] = sa[:, rows]
        w2s[B:] = -ca[:, rows]
        in_maps.append(
            {"mask": m_host, "cst": cst_host, "w2": w2, "w2s": w2s}
        )
    return in_maps


def combine(outs, coupling_mask):
    o = np.stack(outs).astype(np.float64)   # [NCORES, 128, 2*NBLK]
    r = o[:, :, :NBLK].sum(axis=2)          # [NCORES, 128]
    q = o[:, :, NBLK:].sum(axis=2)
    real = (r[:, :B] + r[:, B:]).sum(axis=0)
    imag = (q[:, :B] + q[:, B:]).sum(axis=0)
    n_pairs = max(float(np.asarray(coupling_mask).sum()), 1.0)
    return (np.sqrt(real * real + imag * imag) / n_pairs).astype(np.float32)


_prog_cache: list = []


def kernel(phases_a, phases_b, coupling_mask):
    in_maps = prep_inputs(phases_a, phases_b, coupling_mask)
    if not _prog_cache:
        _prog_cache.append(build_program())
    res = run_bass_kernel_spmd(_prog_cache[0], in_maps, core_ids=list(range(NCORES)))
    return combine([r["out"] for r in res.results], coupling_mask)


# revision 5
# speedup vs baseline: 1.3621x; 1.1255x over previous
"""Masked phase-locking value (PLV) kernel for Trainium2, 8 NeuronCores.

Math: out[b] = |sum_ij M_ij * exp(i*(a_bi - b_bj))| / max(sum(M), 1)
    real_b = sum_ij M_ij (ca_bi cb_bj + sa_bi sb_bj)
    imag_b = sum_ij M_ij (sa_bi cb_bj - ca_bi sb_bj)

Device decomposition (per core, Na sharded 8 ways -> 1024 mask rows each):
    Z[m, i] = sum_j CST[j, m] * maskT[j, i]      (TensorE, contract full Nb)
      where CST[j, m] = cb[m, j] for m<64, sb[m-64, j] for m>=64  (m = 2B = 128)
    racc[m] = sum_i Z[m, i] * W2[m, i]           (DVE scalar_tensor_tensor,
    qacc[m] = sum_i Z[m, i] * W2S[m, i]           fused mul + row-sum accum)
      W2[m,i]  = ca[m,i] | sa[m-64,i],  W2S[m,i] = sa[m,i] | -ca[m-64,i]
real_b = sum_cores racc[b] + racc[64+b]; imag_b = qacc[b] + qacc[64+b].

Contracting Nb (8192) on the PE and only Na/8 (1024) on the DVE makes the
epilogue 8x smaller than the W-stationary orientation. The matmul runs in
fp8 DoubleRow mode (2 fp8 weights per PE cell, 256-deep contraction per
matmul): 64 matmuls of N=512 -> ~15us of PE time, hidden under the ~22us
of DMA (9.25MB/core at ~420GB/s measured). All DMA rides one HWDGE ring in
dependency order (cst trig first -> it gates the first real matmul; w2/w2s
mid-stream before the first epilogue needs them) so the mask stream never
shares bandwidth at the wrong moment. Mask is 0/1 -> exact in fp8e4; trig
in fp8e4 adds ~2e-3 rel err (the coherent real part accumulates quant
noise as a random walk), inside the 2e-2 gate. A PE warm-up burst on a
memset tile during the DMA lead-in defeats the HAM cold-clock penalty.
"""

import numpy as np

import concourse.bass as bass
import concourse.tile as tile
from concourse import bacc, mybir
from concourse.bass_utils import run_bass_kernel_spmd

B = 64
NA = 8192
NB = 8192
NCORES = 8
NASH = NA // NCORES          # mask rows (i) per core
NBLK = 2                     # i blocks per core (PSUM banks)
IBW = NASH // NBLK           # i columns per block = 512
NJC = NB // 256              # 32 DoubleRow matmuls per block (256-deep each)
JCKP = 2 * NJC               # 64 weight half-chunks of 128 j each
NCHUNK = 4                   # mask DMA chunks per i-block (1MB each)
QW = JCKP // NCHUNK          # 16 jckp per DMA chunk

F8 = mybir.dt.float8e4
F16 = mybir.dt.float16
F32 = mybir.dt.float32
DR = mybir.MatmulPerfMode.DoubleRow
MUL = mybir.AluOpType.mult

N_WARM = 8                   # cold-rate N=512 matmuls ~= 3.4us HAM warmup


def build_program() -> bass.Bass:
    nc = bacc.Bacc("TRN2")
    # per-chunk contiguous [p, 16, 512] blocks, chunk-major (ib, q)
    mask_d = nc.dram_tensor("mask", [128 * NBLK * JCKP * IBW], F8, kind="ExternalInput")
    cst_d = nc.dram_tensor("cst", [128 * JCKP * 128], F8, kind="ExternalInput")
    w2_d = nc.dram_tensor("w2", [128, NASH], F8, kind="ExternalInput")
    w2s_d = nc.dram_tensor("w2s", [128, NASH], F8, kind="ExternalInput")
    out_d = nc.dram_tensor("out", [128, 2 * NBLK], F32, kind="ExternalOutput")

    with tile.TileContext(nc) as tc:
        with (
            tc.tile_pool(name="consts", bufs=1) as consts,
            tc.tile_pool(name="scratch", bufs=2) as scratch,
            tc.tile_pool(name="psum", bufs=NBLK, space="PSUM") as psum_pool,
            tc.tile_pool(name="wups", bufs=1, space="PSUM") as wu_pool,
        ):
            # PE warm-up source: memset, no DMA dependency
            wu_in = consts.tile([128, 512], F16)
            nc.vector.memset(wu_in[:], 0.25)

            cst_sb = consts.tile([128, JCKP, 128], F8)
            w2_sb = consts.tile([128, NASH], F8)
            w2s_sb = consts.tile([128, NASH], F8)
            mt = consts.tile([128, NBLK, JCKP, IBW], F8)

            # one HWDGE ring, dependency order: cst gates the first matmul,
            # w2/w2s slot in before the first epilogue, mask fills the rest
            nc.sync.dma_start(
                out=cst_sb[:],
                in_=cst_d[:].rearrange("(p k m) -> p k m", p=128, k=JCKP),
            )
            blk = 128 * QW * IBW

            def mask_chunk(ib, q):
                off = (ib * NCHUNK + q) * blk
                nc.sync.dma_start(
                    out=mt[:, ib, q * QW : (q + 1) * QW, :],
                    in_=mask_d[off : off + blk].rearrange(
                        "(p q i) -> p q i", p=128, q=QW
                    ),
                )

            for q in range(NCHUNK):
                mask_chunk(0, q)
            nc.sync.dma_start(out=w2_sb[:], in_=w2_d[:])
            nc.sync.dma_start(out=w2s_sb[:], in_=w2s_d[:])
            for q in range(NCHUNK):
                mask_chunk(1, q)

            # HAM warm-up while cst + first chunks are in flight
            wu_ps = wu_pool.tile([128, 512], F32)
            for r in range(N_WARM):
                nc.tensor.matmul(
                    out=wu_ps[:],
                    lhsT=wu_in[:, 0:128],
                    rhs=wu_in[:],
                    start=(r == 0),
                    stop=(r == N_WARM - 1),
                )

            racc = consts.tile([128, 2 * NBLK], F32)
            for ib in range(NBLK):
                ps = psum_pool.tile([128, IBW], F32, tag="psum")
                for jc in range(NJC):
                    nc.tensor.matmul(
                        out=ps[:],
                        lhsT=cst_sb[:, 2 * jc : 2 * jc + 2, :],
                        rhs=mt[:, ib, 2 * jc : 2 * jc + 2, :],
                        start=(jc == 0),
                        stop=(jc == NJC - 1),
                        perf_mode=DR,
                    )
                isl = slice(ib * IBW, (ib + 1) * IBW)
                pr = scratch.tile([128, IBW], F32, tag="pr")
                nc.vector.scalar_tensor_tensor(
                    out=pr[:], in0=ps[:], scalar=1.0, in1=w2_sb[:, isl],
                    op0=MUL, op1=MUL, accum_out=racc[:, ib : ib + 1],
                )
                pi = scratch.tile([128, IBW], F32, tag="pr")
                nc.vector.scalar_tensor_tensor(
                    out=pi[:], in0=ps[:], scalar=1.0, in1=w2s_sb[:, isl],
                    op0=MUL, op1=MUL, accum_out=racc[:, NBLK + ib : NBLK + ib + 1],
                )

            nc.scalar.dma_start(out=out_d[:], in_=racc[:])
    nc.finalize()
    return nc


def prep_inputs(phases_a, phases_b, coupling_mask):
    pa = np.asarray(phases_a, dtype=np.float32)
    pb = np.asarray(phases_b, dtype=np.float32)
    ca, sa = np.cos(pa), np.sin(pa)    # (B, Na)
    cb, sb = np.cos(pb), np.sin(pb)    # (B, Nb)
    f8np = mybir.dt.np(F8)

    # CST[j, m]: cb for m<64, sb for m>=64; tile layout [p, jc, kp, m],
    # j = jc*256 + kp*128 + p
    cst = np.concatenate([cb, sb], axis=0).T.astype(f8np)   # (Nb, 128)
    cst_host = np.ascontiguousarray(
        cst.reshape(NJC, 2, 128, 128).transpose(2, 0, 1, 3)
    ).reshape(-1)

    one_byte = np.array([1.0], f8np).view(np.uint8)[0]
    mask_u8 = (np.asarray(coupling_mask) != 0).astype(np.uint8) * one_byte

    in_maps = []
    for c in range(NCORES):
        rows = slice(c * NASH, (c + 1) * NASH)
        # maskT [j, i] -> [p, ib, jckp, i] -> chunk-major [ib, q, p, r, i]
        mT = np.ascontiguousarray(mask_u8[rows].T)          # (Nb, NASH)
        A = mT.reshape(NJC, 2, 128, NBLK, IBW).transpose(2, 3, 0, 1, 4)
        m_host = (
            A.reshape(128, NBLK, NCHUNK, QW, IBW)
            .transpose(1, 2, 0, 3, 4)
            .reshape(-1)
            .view(f8np)
        )

        w2 = np.empty((128, NASH), np.float32)
        w2[:B] = ca[:, rows]
        w2[B:] = sa[:, rows]
        w2s = np.empty((128, NASH), np.float32)
        w2s[:B] = sa[:, rows]
        w2s[B:] = -ca[:, rows]
        in_maps.append(
            {
                "mask": m_host,
                "cst": cst_host,
                "w2": w2.astype(f8np),
                "w2s": w2s.astype(f8np),
            }
        )
    return in_maps


def combine(outs, coupling_mask):
    o = np.stack(outs).astype(np.float64)   # [NCORES, 128, 2*NBLK]
    r = o[:, :, :NBLK].sum(axis=2)          # [NCORES, 128]
    q = o[:, :, NBLK:].sum(axis=2)
    real = (r[:, :B] + r[:, B:]).sum(axis=0)
    imag = (q[:, :B] + q[:, B:]).sum(axis=0)
    n_pairs = max(float(np.asarray(coupling_mask).sum()), 1.0)
    return (np.sqrt(real * real + imag * imag) / n_pairs).astype(np.float32)


_prog_cache: list = []


def kernel(phases_a, phases_b, coupling_mask):
    in_maps = prep_inputs(phases_a, phases_b, coupling_mask)
    if not _prog_cache:
        _prog_cache.append(build_program())
    res = run_bass_kernel_spmd(_prog_cache[0], in_maps, core_ids=list(range(NCORES)))
    return combine([r["out"] for r in res.results], coupling_mask)
